# revision 1
# baseline (speedup 1.0000x reference)
"""Causal attention + output projection on 8 Trainium2 NeuronCores.

Problem (hardcoded): B=2, H=12, T=2048, D=64, DIM=768, fp32.

Sharding: 24 (b, h) pairs -> 3 heads per core; cores 0-3 take b=0,
cores 4-7 take b=1.  Each core computes attention for its 3 heads plus
the partial output projection  sum_h y_h @ W[h*64:(h+1)*64, :]  as a
(T, DIM) partial; the host sums the 4 partials per batch.  No
collectives.

Device-side layout is fully transposed ([s, q]) so no on-chip
transposes are needed:
  - host feeds qT = q^T / sqrt(D) and kT = k^T packed in one tensor
  - host feeds biasT = bias^T with the causal mask pre-added
    (-1e4 on s > q) in bf16 (halves the dominant HBM traffic)
  - v is fed augmented with 64 ones-columns so a single PV matmul
    yields both y^T (rows 0:64) and the softmax denominators
    replicated across rows 64:128.

Per (head, q-chunk of 512, group of 4 s-tiles):
  PSUM[s=128, q=2048] <- identity-matmul copy of biasT (bf16)
  PSUM                += kT-tile.T @ qT-chunk   (fp32, causally trimmed)
  SBUF P = exp(PSUM)                            (one ACT instruction)
  PSUM_y[128, 512]    += vaug-tile.T @ P-slice  (accumulated over s)
then  rec = 1/sums  (DVE, partition-realigning read 64:128 -> 0:64),
      yT[:, chunk] = y_un * rec.
Projection: out[t-block, :] accumulates yT_h-slice.T @ W_h over heads.

Build notes: the program is built on bacc.Bacc and finalize()d —
Bacc.compile()'s generate_event_semaphores pass legalizes multi-wait
instructions for this walrus build (each hardware instruction carries
at most one semaphore wait).  The head loop is a hardware `For_i`
whose back-edge barrier resets all semaphores, so every SBUF slot is
written exactly once per iteration (fresh bias buffer per head) and no
refill DMA carries a slot-release wait.  The exp is issued per PSUM
bank so subtile releases let the next group's matmuls re-enter each
bank as soon as its slice is drained (~9% on the modeled timeline).
"""

import math

import numpy as np
import ml_dtypes

B, H, T, D = 2, 12, 2048, 64
DIM = H * D
NCORES = 8
HPC = 3           # heads per core
P = 128
QC = 512          # q-chunk width (one PSUM bank of fp32)
NJ = T // QC      # 4 q-chunks
NT = T // P       # 16 s-tiles
GROUP = 4         # s-tiles per PSUM logits group (4 banks)

_PROGRAM = None


def _build_program():
    import concourse.bass as bass
    import concourse.mybir as mybir
    import concourse.tile as tile
    from concourse import bacc
    from contextlib import ExitStack

    dt = mybir.dt
    f32 = dt.float32
    bf16 = dt.bfloat16
    EXP = mybir.ActivationFunctionType.Exp
    ds = bass.ds

    nc = bacc.Bacc("TRN2", num_devices=NCORES)
    # flat layouts so per-head slices are register-offset APs
    # per-head fused [va | qT(pad) | kT(pad)] block: one DMA per head
    comb = nc.declare_dram_parameter("comb", [HPC * P, 3 * T], f32, isOutput=False)
    biasT = nc.declare_dram_parameter("biasT", [HPC * 10 * GROUP * P, QC], bf16, isOutput=False)
    wproj = nc.declare_dram_parameter("wproj", [D, HPC * DIM], f32, isOutput=False)
    out = nc.declare_dram_parameter("out", [T, DIM], f32, isOutput=True)

    with tile.TileContext(nc) as tc, ExitStack() as ctx:
        from concourse.masks import make_identity

        const_pool = ctx.enter_context(tc.tile_pool(name="const", bufs=1))
        id_t = const_pool.tile([P, P], bf16)
        make_identity(nc, id_t[:])  # gpsimd memset+affine_select: no DMA lane

        w_pool = ctx.enter_context(tc.tile_pool(name="w", bufs=1))
        w_all = w_pool.tile([D, HPC * DIM], f32)
        nc.sync.dma_start(w_all[:], wproj[:])

        yT_pool = ctx.enter_context(tc.tile_pool(name="yT", bufs=1))
        yT_t = yT_pool.tile([D, HPC * T], f32)

        with (
            tc.tile_pool(name="head", bufs=1) as head_pool,
            tc.tile_pool(name="bias", bufs=1) as bias_pool,
            tc.tile_pool(name="pexp", bufs=2) as pexp_pool,
            tc.tile_pool(name="rec", bufs=2) as rec_pool,
            tc.tile_pool(name="psl", bufs=1, space="PSUM") as psl_pool,
            tc.tile_pool(name="psy", bufs=2, space="PSUM") as psy_pool,
        ):
            with tc.For_i(0, HPC, 1) as hreg:
                cb_t = head_pool.tile([P, 3 * T], f32)
                nc.sync.dma_start(cb_t[:], comb[ds(hreg * P, P), :])
                va_t = cb_t[:, 0:T]
                qT_t = cb_t[0:D, T : 2 * T]
                kT_t = cb_t[0:D, 2 * T : 3 * T]
                NREG = 10
                b_all = bias_pool.tile([P, NREG * GROUP * QC], bf16)
                nc.scalar.dma_start(
                    b_all[:, 0 : 3 * GROUP * QC].rearrange(
                        "p (a q) -> p a q", a=3 * GROUP
                    ),
                    biasT[
                        ds(hreg * (NREG * GROUP * P), 3 * GROUP * P), :
                    ].rearrange("(a p) q -> p a q", p=P),
                )
                nc.scalar.dma_start(
                    b_all[:, 3 * GROUP * QC :].rearrange(
                        "p (a q) -> p a q", a=7 * GROUP
                    ),
                    biasT[
                        ds(hreg * (NREG * GROUP * P) + 3 * GROUP * P,
                           7 * GROUP * P),
                        :,
                    ].rearrange("(a p) q -> p a q", p=P),
                )
                for j in range(NJ):
                    psy_t = psy_pool.tile([P, QC], f32)
                    for g in range(j + 1):
                        r = j * (j + 1) // 2 + g
                        b_t = b_all[:, r * GROUP * QC : (r + 1) * GROUP * QC]
                        psl_t = psl_pool.tile([P, GROUP * QC], f32)
                        for t in range(GROUP):
                            i = g * GROUP + t
                            # bias lands first (identity copy, clears bank)
                            nc.tensor.matmul(
                                psl_t[:, t * QC : (t + 1) * QC],
                                lhsT=id_t[:],
                                rhs=b_t[:, t * QC : (t + 1) * QC],
                                start=True,
                                stop=False,
                            )
                            # causally-trimmed QK accumulate on top
                            c0 = max(0, P * i - QC * j)
                            nc.tensor.matmul(
                                psl_t[:, t * QC + c0 : (t + 1) * QC],
                                lhsT=kT_t[:, i * P : (i + 1) * P],
                                rhs=qT_t[:, j * QC + c0 : (j + 1) * QC],
                                start=False,
                                stop=True,
                            )
                        pe_t = pexp_pool.tile([P, GROUP * QC], f32)
                        # per-bank exp: subtile release lets the next group's
                        # matmuls re-enter each PSUM bank as soon as its
                        # slice is drained, instead of after the whole group
                        for t in range(GROUP):
                            nc.scalar.activation(
                                pe_t[:, t * QC : (t + 1) * QC],
                                psl_t[:, t * QC : (t + 1) * QC],
                                EXP,
                            )
                        for t in range(GROUP):
                            i = g * GROUP + t
                            nc.tensor.matmul(
                                psy_t[:],
                                lhsT=va_t[:, i * P : (i + 1) * P],
                                rhs=pe_t[:, t * QC : (t + 1) * QC],
                                start=(i == 0),
                                stop=(i == 4 * j + 3),
                            )
                    # rows 64:128 of psy hold the softmax denominators
                    # (replicated); realign to partitions 0:64 via the DVE
                    # output crossbar while taking the reciprocal.
                    rec_t = rec_pool.tile([D, QC], f32)
                    nc.vector.reciprocal(rec_t[:], psy_t[D : 2 * D, :])
                    nc.vector.tensor_mul(
                        yT_t[:, ds(hreg * T + j * QC, QC)],
                        psy_t[0:D, :],
                        rec_t[:],
                    )

        with (
            tc.tile_pool(name="psp", bufs=2, space="PSUM") as psp_pool,
            tc.tile_pool(name="outp", bufs=1) as out_pool,
        ):
            o_big = out_pool.tile([P, NT * DIM], f32)
            for tb in range(NT):
                psp_t = psp_pool.tile([P, DIM], f32)
                for o0, ow in ((0, 512), (512, 256)):
                    for h in range(HPC):
                        nc.tensor.matmul(
                            psp_t[:, o0 : o0 + ow],
                            lhsT=yT_t[:, h * T + tb * P : h * T + (tb + 1) * P],
                            rhs=w_all[:, h * DIM + o0 : h * DIM + o0 + ow],
                            start=(h == 0),
                            stop=(h == HPC - 1),
                        )
                nc.vector.tensor_copy(
                    o_big[:, tb * DIM : (tb + 1) * DIM], psp_t[:]
                )
                if tb == NT // 2 - 1:
                    nc.sync.dma_start(
                        out[0 : T // 2, :].rearrange("(a p) o -> p a o", p=P),
                        o_big[:, 0 : (NT // 2) * DIM].rearrange(
                            "p (a o) -> p a o", a=NT // 2
                        ),
                    )
            nc.sync.dma_start(
                out[T // 2 : T, :].rearrange("(a p) o -> p a o", p=P),
                o_big[:, (NT // 2) * DIM :].rearrange(
                    "p (a o) -> p a o", a=NT // 2
                ),
            )

    nc.finalize()
    return nc


def _get_program():
    global _PROGRAM
    if _PROGRAM is None:
        _PROGRAM = _build_program()
    return _PROGRAM


def make_in_maps(q, k, v, attn_bias, W_proj):
    """Host-side sharding/layout prep: one input map per core."""
    q = np.asarray(q, dtype=np.float32)
    k = np.asarray(k, dtype=np.float32)
    v = np.asarray(v, dtype=np.float32)
    attn_bias = np.asarray(attn_bias, dtype=np.float32)
    W_proj = np.asarray(W_proj, dtype=np.float32)

    scale = 1.0 / math.sqrt(D)
    # causal mask in transposed [s, q] coords: masked where s > q
    smask = (np.arange(T)[:, None] > np.arange(T)[None, :]).astype(np.float32)
    smask *= -10000.0
    w_heads = W_proj.reshape(H, D, DIM)

    in_maps = []
    for c in range(NCORES):
        b = c // 4
        h0 = HPC * (c % 4)
        hs = slice(h0, h0 + HPC)
        cb = np.zeros((HPC, P, 3 * T), dtype=np.float32)
        # va blocks: cb[:, :, k*128:(k+1)*128] = [v-tile | ones]
        va = cb[:, :, 0:T].reshape(HPC, P, NT, P)
        va[:, :, :, :D] = v[b, hs].reshape(HPC, NT, P, D).transpose(0, 2, 1, 3)
        va[:, :, :, D:] = 1.0
        cb[:, :D, T : 2 * T] = q[b, hs].transpose(0, 2, 1) * scale
        cb[:, :D, 2 * T : 3 * T] = k[b, hs].transpose(0, 2, 1)
        biasT = attn_bias[b, hs].transpose(0, 2, 1) + smask[None]
        biasT = biasT.astype(ml_dtypes.bfloat16)
        # pack the 10 causally-needed (j-chunk, s-group) regions of each
        # head contiguously: region (j, g) = rows [g*512:(g+1)*512] of
        # column chunk j
        regions = []
        for j in range(NJ):
            for g in range(j + 1):
                regions.append(
                    biasT[:, g * GROUP * P : (g + 1) * GROUP * P,
                          j * QC : (j + 1) * QC]
                )
        biasT = np.ascontiguousarray(
            np.concatenate(regions, axis=1)
        )
        in_maps.append(
            {
                "comb": cb.reshape(HPC * P, 3 * T),
                "biasT": biasT.reshape(HPC * 10 * GROUP * P, QC),
                "wproj": np.ascontiguousarray(
                    w_heads[hs].transpose(1, 0, 2).reshape(D, HPC * DIM)
                ),
            }
        )
    return in_maps


def assemble_output(results):
    """Sum the 4 per-core partial projections for each batch."""
    out = np.zeros((B, T, DIM), dtype=np.float32)
    for c in range(NCORES):
        out[c // 4] += results[c]["out"]
    return out


def kernel(q, k, v, attn_bias, W_proj):
    from concourse.bass_utils import run_bass_kernel_spmd

    nc = _get_program()
    in_maps = make_in_maps(q, k, v, attn_bias, W_proj)
    res = run_bass_kernel_spmd(nc, in_maps, list(range(NCORES)))
    return assemble_output(res.results)



# revision 5
# speedup vs baseline: 1.7312x; 1.7312x over previous
"""Causal attention + output projection on 8 Trainium2 NeuronCores.

Problem (hardcoded): B=2, H=12, T=2048, D=64, DIM=768, fp32.

Sharding: 24 (b, h) pairs -> 3 heads per core; cores 0-3 take b=0,
cores 4-7 take b=1.  Each core computes attention for its 3 heads plus
the partial output projection  sum_h y_h @ W[h*64:(h+1)*64, :]  as a
(T, DIM) partial; the host sums the 4 partials per batch.  No
collectives.

Device-side layout is fully transposed ([s, q]) so no on-chip
transposes are needed:
  - host feeds qT = q^T / sqrt(D) and kT = k^T packed in one tensor
  - host feeds biasT = bias^T with the causal mask pre-added
    (-1e4 on s > q) in bf16 (halves the dominant HBM traffic)
  - v is fed augmented with 64 ones-columns so a single PV matmul
    yields both y^T (rows 0:64) and the softmax denominators
    replicated across rows 64:128.

Per (head, q-chunk of 512, group of 4 s-tiles):
  PSUM[s=128, q=2048] <- identity-matmul copy of biasT (bf16)
  PSUM                += kT-tile.T @ qT-chunk   (fp32, causally trimmed)
  SBUF P = exp(PSUM)                            (one ACT instruction)
  PSUM_y[128, 512]    += vaug-tile.T @ P-slice  (accumulated over s)
then  rec = 1/sums  (DVE, partition-realigning read 64:128 -> 0:64),
      yT[:, chunk] = y_un * rec.
Projection: out[t-block, :] accumulates yT_h-slice.T @ W_h over heads.

Build notes: the program is built on bacc.Bacc and finalize()d —
Bacc.compile()'s generate_event_semaphores pass legalizes multi-wait
instructions for this walrus build (each hardware instruction carries
at most one semaphore wait).  The head loop is a hardware `For_i`
whose back-edge barrier resets all semaphores, so every SBUF slot is
written exactly once per iteration (fresh bias buffer per head) and no
refill DMA carries a slot-release wait.  The exp is issued per PSUM
bank so subtile releases let the next group's matmuls re-enter each
bank as soon as its slice is drained (~9% on the modeled timeline).
"""

import math

import numpy as np
import ml_dtypes

B, H, T, D = 2, 12, 2048, 64
DIM = H * D
NCORES = 8
HPC = 3           # heads per core
P = 128
QC = 512          # q-chunk width (one PSUM bank of fp32)
NJ = T // QC      # 4 q-chunks
NT = T // P       # 16 s-tiles
GROUP = 4         # s-tiles per PSUM logits group (4 banks)

_PROGRAM = None


def _build_program():
    import concourse.bass as bass
    import concourse.mybir as mybir
    import concourse.tile as tile
    from concourse import bacc
    from contextlib import ExitStack

    dt = mybir.dt
    f32 = dt.float32
    bf16 = dt.bfloat16
    EXP = mybir.ActivationFunctionType.Exp
    ds = bass.ds

    nc = bacc.Bacc("TRN2", num_devices=NCORES)
    # flat layouts so per-head slices are register-offset APs
    # per-head fused [va | qT(pad) | kT(pad)] block: one DMA per head
    comb = nc.declare_dram_parameter("comb", [HPC * P, 3 * T], bf16, isOutput=False)
    biasT = nc.declare_dram_parameter("biasT", [HPC * 10 * GROUP * P, QC], bf16, isOutput=False)
    wproj = nc.declare_dram_parameter("wproj", [D, HPC * DIM], bf16, isOutput=False)
    out = nc.declare_dram_parameter("out", [T, DIM], f32, isOutput=True)

    with tile.TileContext(nc) as tc, ExitStack() as ctx:
        from concourse.masks import make_identity

        const_pool = ctx.enter_context(tc.tile_pool(name="const", bufs=1))
        id_t = const_pool.tile([P, P], bf16)
        make_identity(nc, id_t[:])  # gpsimd memset+affine_select: no DMA lane

        w_pool = ctx.enter_context(tc.tile_pool(name="w", bufs=1))
        w_all = w_pool.tile([D, HPC * DIM], bf16)
        nc.sync.dma_start(w_all[:], wproj[:])

        yT_pool = ctx.enter_context(tc.tile_pool(name="yT", bufs=1))
        yT_t = yT_pool.tile([D, HPC * T], bf16)

        with (
            tc.tile_pool(name="head", bufs=2) as head_pool,
            tc.tile_pool(name="bias", bufs=2) as bias_pool,
            tc.tile_pool(name="pexp", bufs=2) as pexp_pool,
            tc.tile_pool(name="rec", bufs=2) as rec_pool,
            tc.tile_pool(name="psl", bufs=1, space="PSUM") as psl_pool,
            tc.tile_pool(name="psy", bufs=2, space="PSUM") as psy_pool,
        ):
            for hreg in range(HPC):
                cb_t = head_pool.tile([P, 3 * T], bf16)
                nc.sync.dma_start(cb_t[:], comb[ds(hreg * P, P), :])
                va_t = cb_t[:, 0:T]
                qT_t = cb_t[0:D, T : 2 * T]
                kT_t = cb_t[0:D, 2 * T : 3 * T]
                NREG = 10
                b_all = bias_pool.tile([P, NREG * GROUP * QC], bf16)
                nc.scalar.dma_start(
                    b_all[:, 0 : 3 * GROUP * QC].rearrange(
                        "p (a q) -> p a q", a=3 * GROUP
                    ),
                    biasT[
                        ds(hreg * (NREG * GROUP * P), 3 * GROUP * P), :
                    ].rearrange("(a p) q -> p a q", p=P),
                )
                nc.scalar.dma_start(
                    b_all[:, 3 * GROUP * QC :].rearrange(
                        "p (a q) -> p a q", a=7 * GROUP
                    ),
                    biasT[
                        ds(hreg * (NREG * GROUP * P) + 3 * GROUP * P,
                           7 * GROUP * P),
                        :,
                    ].rearrange("(a p) q -> p a q", p=P),
                )
                for j in range(NJ):
                    psy_t = psy_pool.tile([P, QC], f32)
                    for g in range(j + 1):
                        r = j * (j + 1) // 2 + g
                        b_t = b_all[:, r * GROUP * QC : (r + 1) * GROUP * QC]
                        psl_t = psl_pool.tile([P, GROUP * QC], f32)
                        for t in range(GROUP):
                            i = g * GROUP + t
                            # bias lands first (identity copy, clears bank)
                            nc.tensor.matmul(
                                psl_t[:, t * QC : (t + 1) * QC],
                                lhsT=id_t[:],
                                rhs=b_t[:, t * QC : (t + 1) * QC],
                                start=True,
                                stop=False,
                            )
                            # causally-trimmed QK accumulate on top
                            c0 = max(0, P * i - QC * j)
                            nc.tensor.matmul(
                                psl_t[:, t * QC + c0 : (t + 1) * QC],
                                lhsT=kT_t[:, i * P : (i + 1) * P],
                                rhs=qT_t[:, j * QC + c0 : (j + 1) * QC],
                                start=False,
                                stop=True,
                            )
                        pe_t = pexp_pool.tile([P, GROUP * QC], bf16)
                        # per-bank exp: subtile release lets the next group's
                        # matmuls re-enter each PSUM bank as soon as its
                        # slice is drained, instead of after the whole group
                        for t in range(GROUP):
                            nc.scalar.activation(
                                pe_t[:, t * QC : (t + 1) * QC],
                                psl_t[:, t * QC : (t + 1) * QC],
                                EXP,
                            )
                        for t in range(GROUP):
                            i = g * GROUP + t
                            nc.tensor.matmul(
                                psy_t[:],
                                lhsT=va_t[:, i * P : (i + 1) * P],
                                rhs=pe_t[:, t * QC : (t + 1) * QC],
                                start=(i == 0),
                                stop=(i == 4 * j + 3),
                            )
                    # rows 64:128 of psy hold the softmax denominators
                    # (replicated); realign to partitions 0:64 via the DVE
                    # output crossbar while taking the reciprocal.
                    rec_t = rec_pool.tile([D, QC], f32)
                    nc.vector.reciprocal(rec_t[:], psy_t[D : 2 * D, :])
                    nc.vector.tensor_mul(
                        yT_t[:, ds(hreg * T + j * QC, QC)],
                        psy_t[0:D, :],
                        rec_t[:],
                    )

        with (
            tc.tile_pool(name="psp", bufs=2, space="PSUM") as psp_pool,
            tc.tile_pool(name="outp", bufs=1) as out_pool,
        ):
            o_big = out_pool.tile([P, NT * DIM], f32)
            for tb in range(NT):
                psp_t = psp_pool.tile([P, DIM], f32)
                for o0, ow in ((0, 512), (512, 256)):
                    for h in range(HPC):
                        nc.tensor.matmul(
                            psp_t[:, o0 : o0 + ow],
                            lhsT=yT_t[:, h * T + tb * P : h * T + (tb + 1) * P],
                            rhs=w_all[:, h * DIM + o0 : h * DIM + o0 + ow],
                            start=(h == 0),
                            stop=(h == HPC - 1),
                        )
                nc.vector.tensor_copy(
                    o_big[:, tb * DIM : (tb + 1) * DIM], psp_t[:]
                )
                if tb == NT // 2 - 1:
                    nc.sync.dma_start(
                        out[0 : T // 2, :].rearrange("(a p) o -> p a o", p=P),
                        o_big[:, 0 : (NT // 2) * DIM].rearrange(
                            "p (a o) -> p a o", a=NT // 2
                        ),
                    )
            nc.sync.dma_start(
                out[T // 2 : T, :].rearrange("(a p) o -> p a o", p=P),
                o_big[:, (NT // 2) * DIM :].rearrange(
                    "p (a o) -> p a o", a=NT // 2
                ),
            )

    nc.finalize()
    return nc


def _get_program():
    global _PROGRAM
    if _PROGRAM is None:
        _PROGRAM = _build_program()
    return _PROGRAM


def make_in_maps(q, k, v, attn_bias, W_proj):
    """Host-side sharding/layout prep: one input map per core."""
    q = np.asarray(q, dtype=np.float32)
    k = np.asarray(k, dtype=np.float32)
    v = np.asarray(v, dtype=np.float32)
    attn_bias = np.asarray(attn_bias, dtype=np.float32)
    W_proj = np.asarray(W_proj, dtype=np.float32)

    scale = 1.0 / math.sqrt(D)
    # causal mask in transposed [s, q] coords: masked where s > q
    smask = (np.arange(T)[:, None] > np.arange(T)[None, :]).astype(np.float32)
    smask *= -10000.0
    w_heads = W_proj.reshape(H, D, DIM)

    in_maps = []
    for c in range(NCORES):
        b = c // 4
        h0 = HPC * (c % 4)
        hs = slice(h0, h0 + HPC)
        cb = np.zeros((HPC, P, 3 * T), dtype=np.float32)
        # va blocks: cb[:, :, k*128:(k+1)*128] = [v-tile | ones]
        va = cb[:, :, 0:T].reshape(HPC, P, NT, P)
        va[:, :, :, :D] = v[b, hs].reshape(HPC, NT, P, D).transpose(0, 2, 1, 3)
        va[:, :, :, D:] = 1.0
        cb[:, :D, T : 2 * T] = q[b, hs].transpose(0, 2, 1) * scale
        cb[:, :D, 2 * T : 3 * T] = k[b, hs].transpose(0, 2, 1)
        biasT = attn_bias[b, hs].transpose(0, 2, 1) + smask[None]
        biasT = biasT.astype(ml_dtypes.bfloat16)
        # pack the 10 causally-needed (j-chunk, s-group) regions of each
        # head contiguously: region (j, g) = rows [g*512:(g+1)*512] of
        # column chunk j
        regions = []
        for j in range(NJ):
            for g in range(j + 1):
                regions.append(
                    biasT[:, g * GROUP * P : (g + 1) * GROUP * P,
                          j * QC : (j + 1) * QC]
                )
        biasT = np.ascontiguousarray(
            np.concatenate(regions, axis=1)
        )
        in_maps.append(
            {
                "comb": cb.reshape(HPC * P, 3 * T).astype(ml_dtypes.bfloat16),
                "biasT": biasT.reshape(HPC * 10 * GROUP * P, QC),
                "wproj": np.ascontiguousarray(
                    w_heads[hs].transpose(1, 0, 2).reshape(D, HPC * DIM)
                ).astype(ml_dtypes.bfloat16),
            }
        )
    return in_maps


def assemble_output(results):
    """Sum the 4 per-core partial projections for each batch."""
    out = np.zeros((B, T, DIM), dtype=np.float32)
    for c in range(NCORES):
        out[c // 4] += results[c]["out"]
    return out


def kernel(q, k, v, attn_bias, W_proj):
    from concourse.bass_utils import run_bass_kernel_spmd

    nc = _get_program()
    in_maps = make_in_maps(q, k, v, attn_bias, W_proj)
    res = run_bass_kernel_spmd(nc, in_maps, list(range(NCORES)))
    return assemble_output(res.results)



# revision 10
# speedup vs baseline: 2.1912x; 1.2658x over previous
"""Causal attention + output projection on 8 Trainium2 NeuronCores.

Problem (hardcoded): B=2, H=12, T=2048, D=64, DIM=768, fp32.

Sharding: 24 (b, h) pairs -> 3 heads per core; cores 0-3 take b=0,
cores 4-7 take b=1.  Each core computes attention for its 3 heads plus
the partial output projection  sum_h y_h @ W[h*64:(h+1)*64, :]  as a
(T, DIM) partial; the host sums the 4 partials per batch.  No
collectives.

Device-side layout is fully transposed ([s, q]) so no on-chip
transposes are needed:
  - host feeds qT = q^T / sqrt(D) and kT = k^T packed in one tensor
  - host feeds biasT = bias^T with the causal mask pre-added
    (-1e4 on s > q) in bf16 (halves the dominant HBM traffic)
  - v is fed augmented with 64 ones-columns so a single PV matmul
    yields both y^T (rows 0:64) and the softmax denominators
    replicated across rows 64:128.

Per (head, q-chunk of 512, group of 4 s-tiles):
  PSUM[s=128, q=2048] <- identity-matmul copy of biasT (bf16)
  PSUM                += kT-tile.T @ qT-chunk   (fp32, causally trimmed)
  SBUF P = exp(PSUM)                            (one ACT instruction)
  PSUM_y[128, 512]    += vaug-tile.T @ P-slice  (accumulated over s)
then  rec = 1/sums  (DVE, partition-realigning read 64:128 -> 0:64),
      yT[:, chunk] = y_un * rec.
Projection: out[t-block, :] accumulates yT_h-slice.T @ W_h over heads.

Build notes: the program is built on bacc.Bacc and finalize()d —
Bacc.compile()'s generate_event_semaphores pass legalizes multi-wait
instructions for this walrus build (each hardware instruction carries
at most one semaphore wait).  The head loop is a hardware `For_i`
whose back-edge barrier resets all semaphores, so every SBUF slot is
written exactly once per iteration (fresh bias buffer per head) and no
refill DMA carries a slot-release wait.  The exp is issued per PSUM
bank so subtile releases let the next group's matmuls re-enter each
bank as soon as its slice is drained (~9% on the modeled timeline).
"""

import math

import numpy as np
import ml_dtypes

B, H, T, D = 2, 12, 2048, 64
DIM = H * D
NCORES = 8
HPC = 3           # heads per core
P = 128
QC = 512          # q-chunk width (one PSUM bank of fp32)
NJ = T // QC      # 4 q-chunks
NT = T // P       # 16 s-tiles
GROUP = 4         # s-tiles per PSUM logits group (4 banks)

_PROGRAM = None


def _build_program():
    import concourse.bass as bass
    import concourse.mybir as mybir
    import concourse.tile as tile
    from concourse import bacc
    from contextlib import ExitStack

    dt = mybir.dt
    f32 = dt.float32
    bf16 = dt.bfloat16
    EXP = mybir.ActivationFunctionType.Exp
    ds = bass.ds

    nc = bacc.Bacc("TRN2", num_devices=NCORES)
    # flat layouts so per-head slices are register-offset APs
    # per-head fused [va | qT(pad) | kT(pad)] block: one DMA per head
    comb = nc.declare_dram_parameter("comb", [HPC * P, 3 * T], bf16, isOutput=False)
    biasT = nc.declare_dram_parameter("biasT", [HPC * 10 * GROUP * P, QC], bf16, isOutput=False)
    wproj = nc.declare_dram_parameter("wproj", [D, HPC * DIM], bf16, isOutput=False)
    out = nc.declare_dram_parameter("out", [T, DIM], bf16, isOutput=True)

    with tile.TileContext(nc) as tc, ExitStack() as ctx:
        from concourse.masks import make_identity

        const_pool = ctx.enter_context(tc.tile_pool(name="const", bufs=1))
        id_t = const_pool.tile([P, P], bf16)
        make_identity(nc, id_t[:])  # gpsimd memset+affine_select: no DMA lane

        w_pool = ctx.enter_context(tc.tile_pool(name="w", bufs=1))
        w_all = w_pool.tile([D, HPC * DIM], bf16)
        nc.gpsimd.dma_start(w_all[:], wproj[:])

        yT_pool = ctx.enter_context(tc.tile_pool(name="yT", bufs=1))
        yT_t = yT_pool.tile([D, HPC * T], bf16)

        with (
            tc.tile_pool(name="head", bufs=2) as head_pool,
            tc.tile_pool(name="bias", bufs=2) as bias_pool,
            tc.tile_pool(name="pexp", bufs=2) as pexp_pool,
            tc.tile_pool(name="rec", bufs=2) as rec_pool,
            tc.tile_pool(name="psl", bufs=1, space="PSUM") as psl_pool,
            tc.tile_pool(name="psy", bufs=2, space="PSUM") as psy_pool,
        ):
            for hreg in range(HPC):
                cb_t = head_pool.tile([P, 3 * T], bf16)
                nc.sync.dma_start(cb_t[:], comb[ds(hreg * P, P), :])
                va_t = cb_t[:, 0:T]
                qT_t = cb_t[0:D, T : 2 * T]
                kT_t = cb_t[0:D, 2 * T : 3 * T]
                NREG = 10
                b_all = bias_pool.tile([P, NREG * GROUP * QC], bf16)
                nc.sync.dma_start(
                    b_all[:, 0 : 3 * GROUP * QC].rearrange(
                        "p (a q) -> p a q", a=3 * GROUP
                    ),
                    biasT[
                        ds(hreg * (NREG * GROUP * P), 3 * GROUP * P), :
                    ].rearrange("(a p) q -> p a q", p=P),
                )
                nc.gpsimd.dma_start(
                    b_all[:, 3 * GROUP * QC :].rearrange(
                        "p (a q) -> p a q", a=7 * GROUP
                    ),
                    biasT[
                        ds(hreg * (NREG * GROUP * P) + 3 * GROUP * P,
                           7 * GROUP * P),
                        :,
                    ].rearrange("(a p) q -> p a q", p=P),
                )
                for j in range(NJ):
                    psy_t = psy_pool.tile([P, QC], f32)
                    for g in range(j + 1):
                        r = j * (j + 1) // 2 + g
                        b_t = b_all[:, r * GROUP * QC : (r + 1) * GROUP * QC]
                        psl_t = psl_pool.tile([P, GROUP * QC], f32)
                        for t in range(GROUP):
                            i = g * GROUP + t
                            # bias lands first (identity copy, clears bank)
                            nc.tensor.matmul(
                                psl_t[:, t * QC : (t + 1) * QC],
                                lhsT=id_t[:],
                                rhs=b_t[:, t * QC : (t + 1) * QC],
                                start=True,
                                stop=False,
                            )
                            # causally-trimmed QK accumulate on top
                            c0 = max(0, P * i - QC * j)
                            nc.tensor.matmul(
                                psl_t[:, t * QC + c0 : (t + 1) * QC],
                                lhsT=kT_t[:, i * P : (i + 1) * P],
                                rhs=qT_t[:, j * QC + c0 : (j + 1) * QC],
                                start=False,
                                stop=True,
                            )
                        pe_t = pexp_pool.tile([P, GROUP * QC], bf16)
                        # per-bank exp: subtile release lets the next group's
                        # matmuls re-enter each PSUM bank as soon as its
                        # slice is drained, instead of after the whole group
                        for t in range(GROUP):
                            nc.scalar.activation(
                                pe_t[:, t * QC : (t + 1) * QC],
                                psl_t[:, t * QC : (t + 1) * QC],
                                EXP,
                            )
                        for t in range(GROUP):
                            i = g * GROUP + t
                            nc.tensor.matmul(
                                psy_t[:],
                                lhsT=va_t[:, i * P : (i + 1) * P],
                                rhs=pe_t[:, t * QC : (t + 1) * QC],
                                start=(i == 0),
                                stop=(i == 4 * j + 3),
                            )
                    # rows 64:128 of psy hold the softmax denominators
                    # (replicated); realign to partitions 0:64 via the DVE
                    # output crossbar while taking the reciprocal.
                    rec_t = rec_pool.tile([D, QC], f32)
                    nc.vector.reciprocal(rec_t[:], psy_t[D : 2 * D, :])
                    nc.vector.tensor_mul(
                        yT_t[:, ds(hreg * T + j * QC, QC)],
                        psy_t[0:D, :],
                        rec_t[:],
                    )

        with (
            tc.tile_pool(name="psp", bufs=2, space="PSUM") as psp_pool,
            tc.tile_pool(name="outp", bufs=1) as out_pool,
        ):
            OB = 4  # tb-blocks per output DMA chunk
            o_big = out_pool.tile([P, NT * DIM], bf16)
            for tb in range(NT):
                psp_t = psp_pool.tile([P, DIM], f32)
                for o0, ow in ((0, 512), (512, 256)):
                    for h in range(HPC):
                        nc.tensor.matmul(
                            psp_t[:, o0 : o0 + ow],
                            lhsT=yT_t[:, h * T + tb * P : h * T + (tb + 1) * P],
                            rhs=w_all[:, h * DIM + o0 : h * DIM + o0 + ow],
                            start=(h == 0),
                            stop=(h == HPC - 1),
                        )
                nc.vector.tensor_copy(
                    o_big[:, tb * DIM : (tb + 1) * DIM], psp_t[:]
                )
                if tb % OB == OB - 1:
                    c0 = tb + 1 - OB
                    eng = nc.sync if (tb // OB) % 2 == 0 else nc.gpsimd
                    eng.dma_start(
                        out[c0 * P : (tb + 1) * P, :].rearrange(
                            "(a p) o -> p a o", p=P
                        ),
                        o_big[:, c0 * DIM : (tb + 1) * DIM].rearrange(
                            "p (a o) -> p a o", a=OB
                        ),
                    )

    nc.finalize()
    return nc


def _get_program():
    global _PROGRAM
    if _PROGRAM is None:
        _PROGRAM = _build_program()
    return _PROGRAM


def make_in_maps(q, k, v, attn_bias, W_proj):
    """Host-side sharding/layout prep: one input map per core."""
    q = np.asarray(q, dtype=np.float32)
    k = np.asarray(k, dtype=np.float32)
    v = np.asarray(v, dtype=np.float32)
    attn_bias = np.asarray(attn_bias, dtype=np.float32)
    W_proj = np.asarray(W_proj, dtype=np.float32)

    scale = 1.0 / math.sqrt(D)
    # causal mask in transposed [s, q] coords: masked where s > q
    smask = (np.arange(T)[:, None] > np.arange(T)[None, :]).astype(np.float32)
    smask *= -10000.0
    w_heads = W_proj.reshape(H, D, DIM)

    in_maps = []
    for c in range(NCORES):
        b = c // 4
        h0 = HPC * (c % 4)
        hs = slice(h0, h0 + HPC)
        cb = np.zeros((HPC, P, 3 * T), dtype=np.float32)
        # va blocks: cb[:, :, k*128:(k+1)*128] = [v-tile | ones]
        va = cb[:, :, 0:T].reshape(HPC, P, NT, P)
        va[:, :, :, :D] = v[b, hs].reshape(HPC, NT, P, D).transpose(0, 2, 1, 3)
        va[:, :, :, D:] = 1.0
        cb[:, :D, T : 2 * T] = q[b, hs].transpose(0, 2, 1) * scale
        cb[:, :D, 2 * T : 3 * T] = k[b, hs].transpose(0, 2, 1)
        biasT = attn_bias[b, hs].transpose(0, 2, 1) + smask[None]
        biasT = biasT.astype(ml_dtypes.bfloat16)
        # pack the 10 causally-needed (j-chunk, s-group) regions of each
        # head contiguously: region (j, g) = rows [g*512:(g+1)*512] of
        # column chunk j
        regions = []
        for j in range(NJ):
            for g in range(j + 1):
                regions.append(
                    biasT[:, g * GROUP * P : (g + 1) * GROUP * P,
                          j * QC : (j + 1) * QC]
                )
        biasT = np.ascontiguousarray(
            np.concatenate(regions, axis=1)
        )
        in_maps.append(
            {
                "comb": cb.reshape(HPC * P, 3 * T).astype(ml_dtypes.bfloat16),
                "biasT": biasT.reshape(HPC * 10 * GROUP * P, QC),
                "wproj": np.ascontiguousarray(
                    w_heads[hs].transpose(1, 0, 2).reshape(D, HPC * DIM)
                ).astype(ml_dtypes.bfloat16),
            }
        )
    return in_maps


def assemble_output(results):
    """Sum the 4 per-core partial projections for each batch."""
    out = np.zeros((B, T, DIM), dtype=np.float32)
    for c in range(NCORES):
        out[c // 4] += np.asarray(results[c]["out"], dtype=np.float32)
    return out


def kernel(q, k, v, attn_bias, W_proj):
    from concourse.bass_utils import run_bass_kernel_spmd

    nc = _get_program()
    in_maps = make_in_maps(q, k, v, attn_bias, W_proj)
    res = run_bass_kernel_spmd(nc, in_maps, list(range(NCORES)))
    return assemble_output(res.results)



# revision 11
# speedup vs baseline: 4.0960x; 1.8693x over previous
"""Causal attention + output projection on 8 Trainium2 NeuronCores.

Problem (hardcoded): B=2, H=12, T=2048, D=64, DIM=768, fp32 in/out.

Sharding: 24 (b, h) pairs -> 3 heads per core; cores 0-3 take b=0,
cores 4-7 take b=1.  Each core computes attention for its 3 heads plus
the partial output projection sum_h y_h @ W[h*64:(h+1)*64, :] as a
(T, DIM) bf16 partial; the host sums the 4 partials per batch in f32.
No collectives.

Device-side layout is fully transposed ([s, q]); all matmul operands
are bf16 (1 PE cycle/row vs 4 for fp32):
  - qkt: [64, 2T] per head = qT/sqrt(D) | kT  (no padding rows)
  - vat: [128, T] per head: 16 blocks of [v-tile | ones] so one PV
    matmul yields y^T (rows 0:64) and softmax denominators (64:128)
  - ebias: exp(bias^T) with exact 0 on causally-masked positions,
    stored trimmed: per (q-chunk j, s-tile i) only cols >= c0 where
    c0 = max(0, 128 i - 512 j).  No bias matmuls: QK writes PSUM with
    start=True and DVE multiplies exp(logits) * ebias in its 4x bf16
    mode.  exp of PSUM cols < c0 reads stale-but-finite logits and is
    never consumed.

Per (head, q-chunk j, group of G=2 s-tiles): QK matmuls (trimmed) ->
one exp (trimmed per-bank on diagonal groups) -> DVE mult -> PV
matmuls (trimmed) accumulating into psy; then reciprocal+normalize
into yT.  Heads 0,1 pack into one [128, T] yT tile (h1 on partitions
64:128) so the projection contracts 128 deep; head 2 is [64, T].
Projection chunks interleave into head 2's stream; out is staged bf16
and DMA'd in 4-block chunks alternating SP/Pool queues.

All DMAs are issued from SP/gpsimd queues only: in this cost model a
DMA blocks its issuing engine's sequencer for the whole transfer, and
transfers serialize globally at ~360 GB/s, so ACT/PE/DVE must stay
clear of DMA duty.
"""

import math

import numpy as np
import ml_dtypes

B, H, T, D = 2, 12, 2048, 64
DIM = H * D
NCORES = 8
HPC = 3           # heads per core
P = 128
QC = 512          # q-chunk width
NJ = T // QC      # 4 q-chunks
NT = T // P       # 16 s-tiles
G = 2             # s-tiles per PSUM logits group

# causal trim tables: chunk j, s-tile i -> start col c0, width w
_C0 = {}
_W = {}
_BOFF = {}        # (j, i) -> col offset of trimmed tile in ebias row
_CHUNK_OFF = {}   # j -> start col of chunk j's region
_acc = 0
for _j in range(NJ):
    _CHUNK_OFF[_j] = _acc
    for _i in range(4 * (_j + 1)):
        _c0 = max(0, P * _i - QC * _j)
        _C0[(_j, _i)] = _c0
        _W[(_j, _i)] = QC - _c0
        _BOFF[(_j, _i)] = _acc
        _acc += QC - _c0
SUMW = _acc       # 17408 trimmed bias cols per head
_CHUNK_OFF[NJ] = SUMW

_PROGRAM = None


def _build_program():
    import concourse.bass as bass
    import concourse.mybir as mybir
    import concourse.tile as tile
    from concourse import bacc
    from contextlib import ExitStack

    dt = mybir.dt
    f32 = dt.float32
    bf16 = dt.bfloat16
    EXP = mybir.ActivationFunctionType.Exp
    ds = bass.ds

    nc = bacc.Bacc("TRN2", num_devices=NCORES)
    vat = nc.declare_dram_parameter("vat", [HPC * P, T], bf16, isOutput=False)
    qkt = nc.declare_dram_parameter("qkt", [HPC * D, 2 * T], bf16, isOutput=False)
    ebias = nc.declare_dram_parameter("ebias", [HPC * P, SUMW], bf16, isOutput=False)
    wproj = nc.declare_dram_parameter("wproj", [P, 2 * DIM], bf16, isOutput=False)
    out = nc.declare_dram_parameter("out", [T, DIM], bf16, isOutput=True)

    with tile.TileContext(nc) as tc, ExitStack() as ctx:
        w_pool = ctx.enter_context(tc.tile_pool(name="w", bufs=1))
        w2 = w_pool.tile([P, 2 * DIM], bf16)

        yT_pool = ctx.enter_context(tc.tile_pool(name="yT", bufs=1))
        yT2 = yT_pool.tile([P, T], bf16)   # heads 0,1 (h1 on partitions 64:)
        yTs = yT_pool.tile([D, T], bf16)   # head 2

        out_pool = ctx.enter_context(tc.tile_pool(name="outp", bufs=1))
        o_big = out_pool.tile([P, NT * DIM], bf16)

        with (
            tc.tile_pool(name="va", bufs=2) as va_pool,
            tc.tile_pool(name="qk", bufs=2) as qk_pool,
            tc.tile_pool(name="eb", bufs=2) as eb_pool,
            tc.tile_pool(name="pexp", bufs=2) as pexp_pool,
            tc.tile_pool(name="pmul", bufs=2) as pmul_pool,
            tc.tile_pool(name="rec", bufs=2) as rec_pool,
            tc.tile_pool(name="psl", bufs=2, space="PSUM") as psl_pool,
            tc.tile_pool(name="psy", bufs=2, space="PSUM") as psy_pool,
            tc.tile_pool(name="psp", bufs=2, space="PSUM") as psp_pool,
        ):
            for hreg in range(HPC):
                qk_t = qk_pool.tile([D, 2 * T], bf16)
                nc.sync.dma_start(qk_t[:], qkt[ds(hreg * D, D), :])
                va_t = va_pool.tile([P, T], bf16)
                nc.sync.dma_start(va_t[:], vat[ds(hreg * P, P), :])
                b_all = eb_pool.tile([P, SUMW], bf16)
                for j in range(NJ):
                    o0, o1 = _CHUNK_OFF[j], _CHUNK_OFF[j + 1]
                    eng = nc.sync if j < 2 else nc.gpsimd
                    eng.dma_start(
                        b_all[:, o0:o1], ebias[ds(hreg * P, P), o0:o1]
                    )
                if hreg == 1:
                    nc.gpsimd.dma_start(w2[:], wproj[:])
                qT = qk_t[:, 0:T]
                kT = qk_t[:, T : 2 * T]

                for j in range(NJ):
                    ntj = 4 * (j + 1)
                    psy_t = psy_pool.tile([P, QC], f32)
                    for g in range(ntj // G):
                        tiles = list(range(g * G, (g + 1) * G))
                        full = all(_C0[(j, i)] == 0 for i in tiles)
                        psl_t = psl_pool.tile([P, G * QC], f32)
                        for t, i in enumerate(tiles):
                            c0 = _C0[(j, i)]
                            nc.tensor.matmul(
                                psl_t[:, t * QC + c0 : (t + 1) * QC],
                                lhsT=kT[:, i * P : (i + 1) * P],
                                rhs=qT[:, j * QC + c0 : (j + 1) * QC],
                                start=True,
                                stop=True,
                            )
                        pe_t = pexp_pool.tile([P, G * QC], bf16)
                        if full:
                            nc.scalar.activation(pe_t[:], psl_t[:], EXP)
                        else:
                            for t, i in enumerate(tiles):
                                c0 = _C0[(j, i)]
                                nc.scalar.activation(
                                    pe_t[:, t * QC + c0 : (t + 1) * QC],
                                    psl_t[:, t * QC + c0 : (t + 1) * QC],
                                    EXP,
                                )
                        pm_t = pmul_pool.tile([P, G * QC], bf16)
                        if full:
                            b0 = _BOFF[(j, tiles[0])]
                            nc.vector.tensor_mul(
                                pm_t[:], pe_t[:], b_all[:, b0 : b0 + G * QC]
                            )
                        else:
                            for t, i in enumerate(tiles):
                                c0, w, b0 = _C0[(j, i)], _W[(j, i)], _BOFF[(j, i)]
                                nc.vector.tensor_mul(
                                    pm_t[:, t * QC + c0 : (t + 1) * QC],
                                    pe_t[:, t * QC + c0 : (t + 1) * QC],
                                    b_all[:, b0 : b0 + w],
                                )
                        for t, i in enumerate(tiles):
                            c0 = _C0[(j, i)]
                            nc.tensor.matmul(
                                psy_t[:, c0:QC],
                                lhsT=va_t[:, i * P : (i + 1) * P],
                                rhs=pm_t[:, t * QC + c0 : (t + 1) * QC],
                                start=(i == 0),
                                stop=(i == ntj - 1),
                            )
                    # rows 64:128 of psy hold the softmax denominators
                    rec_t = rec_pool.tile([D, QC], f32)
                    nc.vector.reciprocal(rec_t[:], psy_t[D : 2 * D, :])
                    if hreg == 0:
                        ydst = yT2[0:D, ds(j * QC, QC)]
                    elif hreg == 1:
                        ydst = yT2[D : 2 * D, ds(j * QC, QC)]
                    else:
                        ydst = yTs[:, ds(j * QC, QC)]
                    nc.vector.tensor_mul(ydst, psy_t[0:D, :], rec_t[:])

                    # interleave projection of the previous chunk into
                    # head 2's stream (all heads' yT ready by then)
                    if hreg == HPC - 1 and j > 0:
                        _proj_chunk(nc, tc, j - 1, yT2, yTs, w2, o_big,
                                    psp_pool, out, f32)
            _proj_chunk(nc, tc, NJ - 1, yT2, yTs, w2, o_big,
                        psp_pool, out, f32)

    nc.finalize()
    return nc


def _proj_chunk(nc, tc, j, yT2, yTs, w2, o_big, psp_pool, out, f32):
    """Projection for q-chunk j: out[tb blocks 4j..4j+4) = y @ W."""
    import concourse.bass as bass

    for tb in range(4 * j, 4 * (j + 1)):
        for o0, ow in ((0, 512), (512, 256)):
            psp_t = psp_pool.tile([P, ow], f32)
            nc.tensor.matmul(
                psp_t[:],
                lhsT=yT2[:, tb * P : (tb + 1) * P],
                rhs=w2[:, o0 : o0 + ow],
                start=True,
                stop=False,
            )
            nc.tensor.matmul(
                psp_t[:],
                lhsT=yTs[:, tb * P : (tb + 1) * P],
                rhs=w2[0:D, DIM + o0 : DIM + o0 + ow],
                start=False,
                stop=True,
            )
            nc.vector.tensor_copy(
                o_big[:, tb * DIM + o0 : tb * DIM + o0 + ow], psp_t[:]
            )
    c0 = 4 * j
    eng = nc.sync if j % 2 == 0 else nc.gpsimd
    eng.dma_start(
        out[c0 * P : (c0 + 4) * P, :].rearrange("(a p) o -> p a o", p=P),
        o_big[:, c0 * DIM : (c0 + 4) * DIM].rearrange("p (a o) -> p a o", a=4),
    )


def _get_program():
    global _PROGRAM
    if _PROGRAM is None:
        _PROGRAM = _build_program()
    return _PROGRAM


def make_in_maps(q, k, v, attn_bias, W_proj):
    """Host-side sharding/layout prep: one input map per core."""
    q = np.asarray(q, dtype=np.float32)
    k = np.asarray(k, dtype=np.float32)
    v = np.asarray(v, dtype=np.float32)
    attn_bias = np.asarray(attn_bias, dtype=np.float32)
    W_proj = np.asarray(W_proj, dtype=np.float32)

    scale = 1.0 / math.sqrt(D)
    # causal mask in transposed [s, q] coords: masked (zeroed) where s > q
    smask = np.arange(T)[:, None] > np.arange(T)[None, :]
    w_heads = W_proj.reshape(H, D, DIM)

    in_maps = []
    for c in range(NCORES):
        b = c // 4
        h0 = HPC * (c % 4)
        hs = slice(h0, h0 + HPC)

        # vat: per head [128, T]: 16 blocks of [v-tile(128x64) | ones]
        va = np.empty((HPC, P, NT, P), dtype=np.float32)
        va[:, :, :, :D] = v[b, hs].reshape(HPC, NT, P, D).transpose(0, 2, 1, 3)
        va[:, :, :, D:] = 1.0

        # qkt: per head [64, 2T] = qT*scale | kT
        qk = np.empty((HPC, D, 2 * T), dtype=np.float32)
        qk[:, :, 0:T] = q[b, hs].transpose(0, 2, 1) * scale
        qk[:, :, T:] = k[b, hs].transpose(0, 2, 1)

        # ebias: exp(bias^T) with exact causal zeros, trimmed pack
        ebias_full = np.exp(attn_bias[b, hs].transpose(0, 2, 1))
        ebias_full[:, smask] = 0.0
        eb = np.empty((HPC, P, SUMW), dtype=np.float32)
        for j in range(NJ):
            for i in range(4 * (j + 1)):
                c0, b0 = _C0[(j, i)], _BOFF[(j, i)]
                eb[:, :, b0 : b0 + QC - c0] = ebias_full[
                    :, i * P : (i + 1) * P, j * QC + c0 : (j + 1) * QC
                ]

        w2 = np.zeros((P, 2 * DIM), dtype=np.float32)
        w2[0:D, 0:DIM] = w_heads[h0]
        w2[D:P, 0:DIM] = w_heads[h0 + 1]
        w2[0:D, DIM:] = w_heads[h0 + 2]

        in_maps.append(
            {
                "vat": va.reshape(HPC * P, T).astype(ml_dtypes.bfloat16),
                "qkt": qk.reshape(HPC * D, 2 * T).astype(ml_dtypes.bfloat16),
                "ebias": eb.reshape(HPC * P, SUMW).astype(ml_dtypes.bfloat16),
                "wproj": w2.astype(ml_dtypes.bfloat16),
            }
        )
    return in_maps


def assemble_output(results):
    """Sum the 4 per-core partial projections for each batch."""
    out = np.zeros((B, T, DIM), dtype=np.float32)
    for c in range(NCORES):
        out[c // 4] += np.asarray(results[c]["out"], dtype=np.float32)
    return out


def kernel(q, k, v, attn_bias, W_proj):
    from concourse.bass_utils import run_bass_kernel_spmd

    nc = _get_program()
    in_maps = make_in_maps(q, k, v, attn_bias, W_proj)
    res = run_bass_kernel_spmd(nc, in_maps, list(range(NCORES)))
    return assemble_output(res.results)


# revision 22
# speedup vs baseline: 4.3886x; 1.0715x over previous
"""Causal attention + output projection on 8 Trainium2 NeuronCores.

Problem (hardcoded): B=2, H=12, T=2048, D=64, DIM=768, fp32 in/out.

Sharding: 24 (b, h) pairs -> 3 heads per core; cores 0-3 take b=0,
cores 4-7 take b=1.  Each core computes attention for its 3 heads plus
the partial output projection sum_h y_h @ W[h*64:(h+1)*64, :] as a
(T, DIM) bf16 partial; the host sums the 4 partials per batch in f32.
No collectives.

Device-side layout is fully transposed ([s, q]); all matmul operands
are bf16 (1 PE cycle/row vs 4 for fp32):
  - qkt: [64, 2T] per head = qT/sqrt(D) | kT  (no padding rows)
  - vat: [128, T] per head: 16 blocks of [v-tile | ones] so one PV
    matmul yields y^T (rows 0:64) and softmax denominators (64:128)
  - ebias: exp(bias^T) with exact 0 on causally-masked positions,
    stored trimmed: per (q-chunk j, s-tile i) only cols >= c0 where
    c0 = max(0, 128 i - 512 j).  No bias matmuls: QK writes PSUM with
    start=True and DVE multiplies exp(logits) * ebias (2x bf16 mode).

Loop structure: q-chunk OUTER, head INNER, so the projection of chunk
j-1 (which needs all heads) interleaves between chunk j's heads and
the DVE/Pool/PE load stays even across the whole timeline.  Per
(chunk j, head, group of G=2 s-tiles): QK matmuls (trimmed) -> exp
(trimmed per-bank on diagonal groups) -> DVE mult -> PV matmuls
(trimmed) into psy; then reciprocal+normalize into yT.  Heads 0,1
pack one [128, T] yT tile (h1 on partitions 64:128) so projection
contracts 128 deep; head 2 is [64, T].  Projection output stages
through PSUM -> bf16 SBUF (gpsimd copies) -> HBM.

All DMAs issue from SP/gpsimd queues only: a DMA blocks its issuing
engine's sequencer for the whole transfer in this cost model, and
transfers serialize globally at ~360 GB/s, so ACT/PE/DVE stay clear
of DMA duty.  First-chunk operands are split into small leading DMAs
so the first QK starts ~1us in.
"""

import math

import numpy as np
import ml_dtypes

B, H, T, D = 2, 12, 2048, 64
DIM = H * D
NCORES = 8
HPC = 3           # heads per core
P = 128
QC = 512          # q-chunk width
NJ = T // QC      # 4 q-chunks
NT = T // P       # 16 s-tiles
G = 2             # s-tiles per PSUM logits group

# causal trim tables: chunk j, s-tile i -> start col c0, width w
_C0 = {}
_W = {}
_BOFF = {}        # (j, i) -> col offset of trimmed tile in ebias row
_CHUNK_OFF = {}   # j -> start col of chunk j's region
_acc = 0
for _j in range(NJ):
    _CHUNK_OFF[_j] = _acc
    for _i in range(4 * (_j + 1)):
        _c0 = max(0, P * _i - QC * _j)
        _C0[(_j, _i)] = _c0
        _W[(_j, _i)] = QC - _c0
        _BOFF[(_j, _i)] = _acc
        _acc += QC - _c0
SUMW = _acc       # 17408 trimmed bias cols per head
_CHUNK_OFF[NJ] = SUMW
WMAX = _CHUNK_OFF[NJ] - _CHUNK_OFF[NJ - 1]  # widest chunk region (7424)

_PROGRAM = None


def _build_program():
    import concourse.bass as bass
    import concourse.mybir as mybir
    import concourse.tile as tile
    from concourse import bacc
    from contextlib import ExitStack

    dt = mybir.dt
    f32 = dt.float32
    bf16 = dt.bfloat16
    EXP = mybir.ActivationFunctionType.Exp
    ds = bass.ds

    nc = bacc.Bacc("TRN2", num_devices=NCORES)
    vat = nc.declare_dram_parameter("vat", [HPC * P, T], bf16, isOutput=False)
    qkt = nc.declare_dram_parameter("qkt", [HPC * D, 2 * T], bf16, isOutput=False)
    ebias = nc.declare_dram_parameter("ebias", [HPC * P, SUMW], bf16, isOutput=False)
    wproj = nc.declare_dram_parameter("wproj", [P, 2 * DIM], bf16, isOutput=False)
    out = nc.declare_dram_parameter("out", [T, DIM], bf16, isOutput=True)

    with tile.TileContext(nc) as tc, ExitStack() as ctx:
        pers = ctx.enter_context(tc.tile_pool(name="pers", bufs=1))
        w2 = pers.tile([P, 2 * DIM], bf16)
        qk_t = [pers.tile([D, 2 * T], bf16, name=f"qk{h}") for h in range(HPC)]
        va_t = [pers.tile([P, T], bf16, name=f"va{h}") for h in range(HPC)]
        yT2 = pers.tile([P, T], bf16)   # heads 0,1 (h1 on partitions 64:)
        yTs = pers.tile([D, T], bf16)   # head 2
        o_big = pers.tile([P, NT * DIM], bf16)

        # prologue DMAs: critical first-chunk slices lead, rests follow.
        # SP carries heads 0,1; gpsimd carries head 2 + w.
        def eng(h):
            return nc.sync if h < 2 else nc.gpsimd

        for h in range(HPC):
            e = eng(h)
            e.dma_start(qk_t[h][:, 0:QC], qkt[ds(h * D, D), 0:QC])
            e.dma_start(qk_t[h][:, T : T + QC], qkt[ds(h * D, D), T : T + QC])

        with (
            tc.tile_pool(name="eb", bufs=3) as eb_pool,
            tc.tile_pool(name="pexp", bufs=2) as pexp_pool,
            tc.tile_pool(name="pmul", bufs=2) as pmul_pool,
            tc.tile_pool(name="rec", bufs=2) as rec_pool,
            tc.tile_pool(name="psl", bufs=2, space="PSUM") as psl_pool,
            tc.tile_pool(name="psy", bufs=2, space="PSUM") as psy_pool,
            tc.tile_pool(name="psp", bufs=2, space="PSUM") as psp_pool,
        ):
            eb_t = {}
            for h in range(HPC):
                o0, o1 = _CHUNK_OFF[0], _CHUNK_OFF[1]
                eb_t[(h, 0)] = eb_pool.tile([P, WMAX], bf16, name="ebt")
                eng(h).dma_start(
                    eb_t[(h, 0)][:, 0 : o1 - o0], ebias[ds(h * P, P), o0:o1]
                )
                eng(h).dma_start(va_t[h][:, 0:QC], vat[ds(h * P, P), 0:QC])
            for h in range(HPC):
                e = eng(h)
                e.dma_start(qk_t[h][:, QC:T], qkt[ds(h * D, D), QC:T])
                e.dma_start(qk_t[h][:, T + QC :], qkt[ds(h * D, D), T + QC :])
                e.dma_start(va_t[h][:, QC:], vat[ds(h * P, P), QC:])
            nc.gpsimd.dma_start(w2[:], wproj[:])

            for j in range(NJ):
                ntj = 4 * (j + 1)
                # prefetch next chunk's ebias (bufs=3 throttles lookahead)
                if j + 1 < NJ:
                    o0, o1 = _CHUNK_OFF[j + 1], _CHUNK_OFF[j + 2]
                    for h in range(HPC):
                        tl = eb_pool.tile([P, WMAX], bf16, name="ebt")
                        eb_t[(h, j + 1)] = tl
                        eng(h).dma_start(
                            tl[:, 0 : o1 - o0], ebias[ds(h * P, P), o0:o1]
                        )
                for h in range(HPC):
                    qT = qk_t[h][:, 0:T]
                    kT = qk_t[h][:, T : 2 * T]
                    ebh = eb_t[(h, j)]
                    psy_t = psy_pool.tile([P, QC], f32)
                    for g in range(ntj // G):
                        tiles = list(range(g * G, (g + 1) * G))
                        full = all(_C0[(j, i)] == 0 for i in tiles)
                        psl_t = psl_pool.tile([P, G * QC], f32)
                        for t, i in enumerate(tiles):
                            c0 = _C0[(j, i)]
                            nc.tensor.matmul(
                                psl_t[:, t * QC + c0 : (t + 1) * QC],
                                lhsT=kT[:, i * P : (i + 1) * P],
                                rhs=qT[:, j * QC + c0 : (j + 1) * QC],
                                start=True,
                                stop=True,
                            )
                        pe_t = pexp_pool.tile([P, G * QC], bf16)
                        if full:
                            nc.scalar.activation(pe_t[:], psl_t[:], EXP)
                        else:
                            for t, i in enumerate(tiles):
                                c0 = _C0[(j, i)]
                                nc.scalar.activation(
                                    pe_t[:, t * QC + c0 : (t + 1) * QC],
                                    psl_t[:, t * QC + c0 : (t + 1) * QC],
                                    EXP,
                                )
                        pm_t = pmul_pool.tile([P, G * QC], bf16)
                        if full:
                            b0 = _BOFF[(j, tiles[0])] - _CHUNK_OFF[j]
                            nc.vector.tensor_mul(
                                pm_t[:], pe_t[:], ebh[:, b0 : b0 + G * QC]
                            )
                        else:
                            for t, i in enumerate(tiles):
                                c0, w = _C0[(j, i)], _W[(j, i)]
                                b0 = _BOFF[(j, i)] - _CHUNK_OFF[j]
                                nc.vector.tensor_mul(
                                    pm_t[:, t * QC + c0 : (t + 1) * QC],
                                    pe_t[:, t * QC + c0 : (t + 1) * QC],
                                    ebh[:, b0 : b0 + w],
                                )
                        for t, i in enumerate(tiles):
                            c0 = _C0[(j, i)]
                            nc.tensor.matmul(
                                psy_t[:, c0:QC],
                                lhsT=va_t[h][:, i * P : (i + 1) * P],
                                rhs=pm_t[:, t * QC + c0 : (t + 1) * QC],
                                start=(i == 0),
                                stop=(i == ntj - 1),
                            )
                    # rows 64:128 of psy hold the softmax denominators
                    rec_t = rec_pool.tile([D, QC], f32)
                    nc.vector.reciprocal(rec_t[:], psy_t[D : 2 * D, :])
                    if h == 0:
                        ydst = yT2[0:D, ds(j * QC, QC)]
                    elif h == 1:
                        ydst = yT2[D : 2 * D, ds(j * QC, QC)]
                    else:
                        ydst = yTs[:, ds(j * QC, QC)]
                    nc.vector.tensor_mul(ydst, psy_t[0:D, :], rec_t[:])

                    # interleave projection of the previous chunk: one or
                    # two tb-blocks after each head's attention
                    if j > 0:
                        tbs = ([4 * (j - 1)], [4 * (j - 1) + 1],
                               [4 * (j - 1) + 2, 4 * (j - 1) + 3])[h]
                        for tb in tbs:
                            _proj_tb(nc, tb, yT2, yTs, w2, o_big,
                                     psp_pool, f32)
                        if h == HPC - 1:
                            _out_dma(nc, j - 1, o_big, out)
            for tb in range(4 * (NJ - 1), 4 * NJ):
                _proj_tb(nc, tb, yT2, yTs, w2, o_big, psp_pool, f32)
                if tb % 2 == 1:
                    _out_dma(nc, tb // 2 + 6, o_big, out, nblk=2)

    nc.finalize()
    return nc


def _proj_tb(nc, tb, yT2, yTs, w2, o_big, psp_pool, f32):
    """Projection for one 128-row output block tb."""
    for o0, ow in ((0, 512), (512, 256)):
        psp_t = psp_pool.tile([P, ow], f32, name="psp_t")
        nc.tensor.matmul(
            psp_t[:],
            lhsT=yT2[:, tb * P : (tb + 1) * P],
            rhs=w2[:, o0 : o0 + ow],
            start=True,
            stop=False,
        )
        nc.tensor.matmul(
            psp_t[:],
            lhsT=yTs[:, tb * P : (tb + 1) * P],
            rhs=w2[0:D, DIM + o0 : DIM + o0 + ow],
            start=False,
            stop=True,
        )
        nc.vector.tensor_copy(
            o_big[:, tb * DIM + o0 : tb * DIM + o0 + ow], psp_t[:]
        )


def _out_dma(nc, c, o_big, out, nblk=4):
    """DMA output blocks [c*nblk, (c+1)*nblk) to HBM (c in 4-block units
    when nblk=4; in 2-block units offset by 6 when nblk=2)."""
    b0 = c * 4 if nblk == 4 else (c - 12) * 2 + 12
    eng = nc.sync if c % 2 == 0 else nc.gpsimd
    eng.dma_start(
        out[b0 * P : (b0 + nblk) * P, :].rearrange("(a p) o -> p a o", p=P),
        o_big[:, b0 * DIM : (b0 + nblk) * DIM].rearrange(
            "p (a o) -> p a o", a=nblk
        ),
    )


def _get_program():
    global _PROGRAM
    if _PROGRAM is None:
        _PROGRAM = _build_program()
    return _PROGRAM


def make_in_maps(q, k, v, attn_bias, W_proj):
    """Host-side sharding/layout prep: one input map per core."""
    q = np.asarray(q, dtype=np.float32)
    k = np.asarray(k, dtype=np.float32)
    v = np.asarray(v, dtype=np.float32)
    attn_bias = np.asarray(attn_bias, dtype=np.float32)
    W_proj = np.asarray(W_proj, dtype=np.float32)

    scale = 1.0 / math.sqrt(D)
    # causal mask in transposed [s, q] coords: masked (zeroed) where s > q
    smask = np.arange(T)[:, None] > np.arange(T)[None, :]
    w_heads = W_proj.reshape(H, D, DIM)

    in_maps = []
    for c in range(NCORES):
        b = c // 4
        h0 = HPC * (c % 4)
        hs = slice(h0, h0 + HPC)

        # vat: per head [128, T]: 16 blocks of [v-tile(128x64) | ones]
        va = np.empty((HPC, P, NT, P), dtype=np.float32)
        va[:, :, :, :D] = v[b, hs].reshape(HPC, NT, P, D).transpose(0, 2, 1, 3)
        va[:, :, :, D:] = 1.0

        # qkt: per head [64, 2T] = qT*scale | kT
        qk = np.empty((HPC, D, 2 * T), dtype=np.float32)
        qk[:, :, 0:T] = q[b, hs].transpose(0, 2, 1) * scale
        qk[:, :, T:] = k[b, hs].transpose(0, 2, 1)

        # ebias: exp(bias^T) with exact causal zeros, trimmed pack
        ebias_full = np.exp(attn_bias[b, hs].transpose(0, 2, 1))
        ebias_full[:, smask] = 0.0
        eb = np.empty((HPC, P, SUMW), dtype=np.float32)
        for j in range(NJ):
            for i in range(4 * (j + 1)):
                c0, b0 = _C0[(j, i)], _BOFF[(j, i)]
                eb[:, :, b0 : b0 + QC - c0] = ebias_full[
                    :, i * P : (i + 1) * P, j * QC + c0 : (j + 1) * QC
                ]

        w2 = np.zeros((P, 2 * DIM), dtype=np.float32)
        w2[0:D, 0:DIM] = w_heads[h0]
        w2[D:P, 0:DIM] = w_heads[h0 + 1]
        w2[0:D, DIM:] = w_heads[h0 + 2]

        in_maps.append(
            {
                "vat": va.reshape(HPC * P, T).astype(ml_dtypes.bfloat16),
                "qkt": qk.reshape(HPC * D, 2 * T).astype(ml_dtypes.bfloat16),
                "ebias": eb.reshape(HPC * P, SUMW).astype(ml_dtypes.bfloat16),
                "wproj": w2.astype(ml_dtypes.bfloat16),
            }
        )
    return in_maps


def assemble_output(results):
    """Sum the 4 per-core partial projections for each batch."""
    out = np.zeros((B, T, DIM), dtype=np.float32)
    for c in range(NCORES):
        out[c // 4] += np.asarray(results[c]["out"], dtype=np.float32)
    return out


def kernel(q, k, v, attn_bias, W_proj):
    from concourse.bass_utils import run_bass_kernel_spmd

    nc = _get_program()
    in_maps = make_in_maps(q, k, v, attn_bias, W_proj)
    res = run_bass_kernel_spmd(nc, in_maps, list(range(NCORES)))
    return assemble_output(res.results)
